# revision 10
# baseline (speedup 1.0000x reference)
"""Trainium2 Bass kernel for CLSProcess: diagonal linear recurrence
state_t = y_t * state_{t-1} + x_t * z_t over [B=8, T=4096, units=1024].

Sharding: batch across the 8 cores (one batch element per core).

v3 design (~2x over the v1 baseline):
  - all matmuls bf16 (f32r lowers to 4-pass fp32 "fp32_mode=HIGH" on this
    toolchain: ~755ns per 512-col matmul vs ~213ns bf16). z arrives in
    SBUF already in bf16 via gpsimd SWDGE cast-DMAs (f32->bf16 in
    flight), one DMA per 4 blocks with a "(a b) c -> b (a c)" rearrange
    so partition p holds rows {g*512+p, +128, +256, +384}.
  - x and y never need a transpose: a [128, 8] sideband DMA per group
    (columns 0:2 rearranged) gives per-block x/y columns, and one
    prologue SWDGE gather pulls the whole y row [1, 4096] (block-start
    positions re-zeroed by a strided memset for the scan reset).
  - per block, the decay matrix M[t,s] = prod y is built by a DVE
    tensor_tensor_scan over the identity (batched 4 blocks/scan), then
    x is folded into the bf16 weight download (activation Copy with
    scale=x column): lhsT = x_s*M[t,s].
  - carry term: po[t,:] += p_t * prev[127,:] via a rank-1 bf16 matmul
    with sel[s,t] = I[s==127]*p_t (full K=128: base-64 K=64 tiles
    silently corrupt the accumulation group).
  - output written bf16 (halves output traffic; host upconverts),
    single PSUM drain per block split DVE/Act, two blocks batched per
    output DMA (SP queue) via the same DRAM rearrange.
  - DMA traffic/core: 16.8 MB in + 8.4 MB out = 25.2 MB -> ~70us floor
    at 16 DMA engines x 22.5 B/ns.
"""

import numpy as np

import concourse.bacc as bacc
import concourse.bass as bass
import concourse.mybir as mybir
import concourse.tile as tile
from concourse.bass_utils import run_bass_kernel_spmd

B = 8
T = 4096
F = 1026
U = 1024
L = 128
G = 4  # blocks per group (one scan / one z cast-DMA per group)
OB = 2  # blocks per output DMA
f32 = mybir.dt.float32
f32r = mybir.dt.float32r
bf16 = mybir.dt.bfloat16
Copy = mybir.ActivationFunctionType.Copy


def build_nc(t_total: int = T) -> bass.Bass:
    nb = t_total // L
    ng = nb // G
    nc = bacc.Bacc()
    inp = nc.dram_tensor("inp", [t_total, F], f32, kind="ExternalInput")
    out = nc.dram_tensor("out", [t_total, U], bf16, kind="ExternalOutput")
    ident4_d = nc.inline_tensor(
        np.tile(np.eye(L, dtype=np.float32), (1, G)), name="ident4"
    )
    e127c_np = np.zeros((L, 1), dtype=np.float32)
    e127c_np[L - 1, 0] = 1.0
    e127c_d = nc.inline_tensor(e127c_np, name="e127c")

    with tile.TileContext(nc) as tc:
        with (
            tc.tile_pool(name="const", bufs=1) as constp,
            tc.tile_pool(name="yrow", bufs=1) as yrowp,
            tc.tile_pool(name="zpool", bufs=4) as zpool,
            tc.tile_pool(name="xypool", bufs=4) as xypool,
            tc.tile_pool(name="mpool", bufs=2) as mpool,
            tc.tile_pool(name="mscpool", bufs=4) as mscpool,
            tc.tile_pool(name="rowpool", bufs=2) as rowpool,
            tc.tile_pool(name="bcpool", bufs=3) as bcpool,
            tc.tile_pool(name="pbpool", bufs=2) as pbpool,
            tc.tile_pool(name="selpool", bufs=3) as selpool,
            tc.tile_pool(name="otbpool", bufs=3) as otbpool,
            tc.tile_pool(name="ps_out", bufs=4, space="PSUM") as ps_out_pool,
        ):
            ident4 = constp.tile([L, G * L], f32, tag="ident4")
            nc.sync.dma_start(ident4[:], ident4_d[:, :])
            e127c = constp.tile([L, 1], f32, tag="e127c")
            nc.sync.dma_start(e127c[:], e127c_d[:, :])

            # whole y row, gathered once; block-start positions re-zeroed
            # (strided memset over the 3D view) so the M-scan resets
            yz3 = yrowp.tile([1, nb, L], f32, tag="yz3")
            nc.gpsimd.dma_start(
                yz3[0:1, :, :], inp[:, 1:2].rearrange("a b -> b a")
            )
            nc.gpsimd.memset(yz3[0:1, :, 0:1], 0.0)

            zts = {}
            xys = {}
            ybcs = {}

            def dispatch_group(g: int):
                r0 = g * G * L
                # z for 4 blocks, cast f32->bf16 in the DMA (SWDGE):
                # partition p <- rows {r0+p, r0+128+p, r0+256+p, r0+384+p}
                zt = zpool.tile([L, G * U], bf16, tag="zt")
                nc.gpsimd.dma_start(
                    zt[:],
                    inp[r0 : r0 + G * L, 2:F].rearrange("(a b) c -> b a c", a=G),
                )
                zts[g] = zt
                xy = xypool.tile([L, G * 2], f32, tag="xy")
                nc.sync.dma_start(
                    xy[:],
                    inp[r0 : r0 + G * L, 0:2].rearrange("(a b) c -> b a c", a=G),
                )
                xys[g] = xy
                ybc = bcpool.tile([L, G * L], f32, tag="ybc")
                nc.gpsimd.partition_broadcast(ybc[:], yz3[0:1, G * g : G * (g + 1), :])
                ybcs[g] = ybc

            dispatch_group(0)
            dispatch_group(1)
            prev = None  # (tile, column offset) of previous block's output
            for g in range(ng):
                if g + 2 < ng:
                    dispatch_group(g + 2)
                zt = zts.pop(g)
                xy = xys.pop(g)
                # mt4[s, L*j + t] = M_j[t, s] = prod_{r=s+1..t} y_r
                mt4 = mpool.tile([L, G * L], f32r, tag="mt4")
                nc.vector.tensor_tensor_scan(
                    mt4[:],
                    ybcs.pop(g)[:],
                    ident4[:],
                    0.0,
                    mybir.AluOpType.mult,
                    mybir.AluOpType.add,
                )
                for j in range(G):
                    k = g * G + j
                    mtk = mt4[:, L * j : L * j + L]
                    xcol = xy[:, 2 * j : 2 * j + 1]
                    y0 = xy[0:1, 2 * j + 1 : 2 * j + 2]

                    # bf16 weights: msc[s, t] = x_s * M[t, s]
                    msc = mscpool.tile([L, L], bf16, tag="msc")
                    nc.scalar.activation(msc[:], mtk, Copy, scale=xcol)

                    if k > 0:
                        # p_t = prod_{r=block_start..t} y_r = y_0 * mt[0, t]
                        prow = rowpool.tile([1, L], f32, tag="prow")
                        nc.vector.tensor_scalar_mul(prow[:], mtk[0:1, :], y0)
                        # sel[s, t] = I[s==127] * p_t
                        pb = pbpool.tile([L, L], f32, tag="pb")
                        nc.gpsimd.partition_broadcast(pb[:], prow[0:1, :])
                        sel = selpool.tile([L, L], bf16, tag="sel")
                        nc.scalar.activation(sel[:], pb[:], Copy, scale=e127c[:])

                    po = ps_out_pool.tile([L, U], f32, tag="po")
                    for jj in (0, 512):
                        nc.tensor.matmul(
                            po[:, jj : jj + 512],
                            msc[:],
                            zt[:, j * U + jj : j * U + jj + 512],
                            start=True,
                            stop=(k == 0),
                        )
                    if k > 0:
                        # po[t, :] += p_t * prev[127, :]
                        pt, pc = prev
                        for jj in (0, 512):
                            nc.tensor.matmul(
                                po[:, jj : jj + 512],
                                sel[:],
                                pt[:, pc + jj : pc + jj + 512],
                                start=False,
                                stop=True,
                            )
                    # single bf16 drain, split DVE/Act; OB blocks share one
                    # otb tile -> one batched output DMA
                    h = k % OB
                    if h == 0:
                        otb = otbpool.tile([L, OB * U], bf16, tag="otb")
                    c0 = h * U
                    nc.vector.tensor_copy(otb[:, c0 : c0 + 512], po[:, 0:512])
                    nc.scalar.copy(otb[:, c0 + 512 : c0 + U], po[:, 512:U])
                    prev = (otb, c0)
                    if h == OB - 1:
                        r0b = (k - OB + 1) * L
                        nc.sync.dma_start(
                            out[r0b : r0b + OB * L, :].rearrange("(a b) c -> b a c", a=OB),
                            otb[:],
                        )
    nc.finalize()
    return nc


_NC = None


def _get_nc() -> bass.Bass:
    global _NC
    if _NC is None:
        _NC = build_nc()
    return _NC


def kernel(**inputs: np.ndarray) -> np.ndarray:
    x = np.ascontiguousarray(inputs["inputs"], dtype=np.float32)
    assert x.shape == (B, T, F), x.shape
    nc = _get_nc()
    in_maps = [{"inp": x[c]} for c in range(B)]
    res = run_bass_kernel_spmd(nc, in_maps, core_ids=list(range(B)))
    return np.stack(
        [np.asarray(res.results[c]["out"]).astype(np.float32) for c in range(B)],
        axis=0,
    )


# revision 13
# speedup vs baseline: 1.0429x; 1.0429x over previous
"""Trainium2 Bass kernel for CLSProcess: diagonal linear recurrence
state_t = y_t * state_{t-1} + x_t * z_t over [B=8, T=4096, units=1024].

Sharding: batch across the 8 cores (one batch element per core).

v3 design (~2x over the v1 baseline):
  - all matmuls bf16 (f32r lowers to 4-pass fp32 "fp32_mode=HIGH" on this
    toolchain: ~755ns per 512-col matmul vs ~213ns bf16). z arrives in
    SBUF already in bf16 via gpsimd SWDGE cast-DMAs (f32->bf16 in
    flight), one DMA per 4 blocks with a "(a b) c -> b (a c)" rearrange
    so partition p holds rows {g*512+p, +128, +256, +384}.
  - x and y never need a transpose: a [128, 8] sideband DMA per group
    (columns 0:2 rearranged) gives per-block x/y columns, and one
    prologue SWDGE gather pulls the whole y row [1, 4096] (block-start
    positions re-zeroed by a strided memset for the scan reset).
  - per block, the decay matrix M[t,s] = prod y is built by a DVE
    tensor_tensor_scan over the identity (batched 4 blocks/scan), then
    x is folded into the bf16 weight download (activation Copy with
    scale=x column): lhsT = x_s*M[t,s].
  - carry term: po[t,:] += p_t * prev[127,:] via a rank-1 bf16 matmul
    with sel[s,t] = I[s==127]*p_t (full K=128: base-64 K=64 tiles
    silently corrupt the accumulation group).
  - output written bf16 (halves output traffic; host upconverts),
    single PSUM drain per block split DVE/Act, two blocks batched per
    output DMA (SP queue) via the same DRAM rearrange.
  - DMA traffic/core: 16.8 MB in + 8.4 MB out = 25.2 MB -> ~70us floor
    at 16 DMA engines x 22.5 B/ns.
"""

import numpy as np

import concourse.bacc as bacc
import concourse.bass as bass
import concourse.mybir as mybir
import concourse.tile as tile
from concourse.bass_utils import run_bass_kernel_spmd

B = 8
T = 4096
F = 1026
U = 1024
L = 128
G = 4  # blocks per group (one scan / one z cast-DMA per group)
OB = 2  # blocks per output DMA
f32 = mybir.dt.float32
f32r = mybir.dt.float32r
bf16 = mybir.dt.bfloat16
Copy = mybir.ActivationFunctionType.Copy


def build_nc(t_total: int = T) -> bass.Bass:
    nb = t_total // L
    ng = nb // G
    nc = bacc.Bacc()
    inp = nc.dram_tensor("inp", [t_total, F], f32, kind="ExternalInput")
    out = nc.dram_tensor("out", [t_total, U], bf16, kind="ExternalOutput")
    ident4_d = nc.inline_tensor(
        np.tile(np.eye(L, dtype=np.float32), (1, G)), name="ident4"
    )
    e127c_np = np.zeros((L, 1), dtype=np.float32)
    e127c_np[L - 1, 0] = 1.0
    e127c_d = nc.inline_tensor(e127c_np, name="e127c")

    with tile.TileContext(nc) as tc:
        with (
            tc.tile_pool(name="const", bufs=1) as constp,
            tc.tile_pool(name="yrow", bufs=1) as yrowp,
            tc.tile_pool(name="zpool", bufs=8) as zpool,
            tc.tile_pool(name="xypool", bufs=8) as xypool,
            tc.tile_pool(name="mpool", bufs=2) as mpool,
            tc.tile_pool(name="mscpool", bufs=4) as mscpool,
            tc.tile_pool(name="rowpool", bufs=2) as rowpool,
            tc.tile_pool(name="bcpool", bufs=8) as bcpool,
            tc.tile_pool(name="pbpool", bufs=2) as pbpool,
            tc.tile_pool(name="selpool", bufs=3) as selpool,
            tc.tile_pool(name="otbpool", bufs=3) as otbpool,
            tc.tile_pool(name="ps_out", bufs=4, space="PSUM") as ps_out_pool,
        ):
            ident4 = constp.tile([L, G * L], f32, tag="ident4")
            nc.sync.dma_start(ident4[:], ident4_d[:, :])
            e127c = constp.tile([L, 1], f32, tag="e127c")
            nc.sync.dma_start(e127c[:], e127c_d[:, :])

            # whole y row, gathered once; block-start positions re-zeroed
            # (strided memset over the 3D view) so the M-scan resets
            yz3 = yrowp.tile([1, nb, L], f32, tag="yz3")
            nc.gpsimd.dma_start(
                yz3[0:1, :, :], inp[:, 1:2].rearrange("a b -> b a")
            )
            nc.gpsimd.memset(yz3[0:1, :, 0:1], 0.0)

            zts = {}
            xys = {}
            ybcs = {}

            def dispatch_group(g: int):
                r0 = g * G * L
                # z for 4 blocks, cast f32->bf16 in the DMA (SWDGE):
                # partition p <- rows {r0+p, r0+128+p, r0+256+p, r0+384+p}
                zt = zpool.tile([L, G * U], bf16, tag="zt")
                nc.gpsimd.dma_start(
                    zt[:],
                    inp[r0 : r0 + G * L, 2:F].rearrange("(a b) c -> b a c", a=G),
                )
                zts[g] = zt
                xy = xypool.tile([L, G * 2], f32, tag="xy")
                nc.sync.dma_start(
                    xy[:],
                    inp[r0 : r0 + G * L, 0:2].rearrange("(a b) c -> b a c", a=G),
                )
                xys[g] = xy
                ybc = bcpool.tile([L, G * L], f32, tag="ybc")
                nc.gpsimd.partition_broadcast(ybc[:], yz3[0:1, G * g : G * (g + 1), :])
                ybcs[g] = ybc

            # dispatch ALL input DMAs + y broadcasts upfront: nothing in the
            # steady-state loop ever blocks input prefetch
            for g in range(ng):
                dispatch_group(g)

            pos = {}
            sels = {}
            otbs = {}
            mt4 = None

            def front(k: int):
                # weights + mains for block k (no dependence on block k-1)
                nonlocal mt4
                g, j = divmod(k, G)
                if j == 0:
                    m = mpool.tile([L, G * L], f32r, tag="mt4")
                    nc.vector.tensor_tensor_scan(
                        m[:],
                        ybcs.pop(g)[:],
                        ident4[:],
                        0.0,
                        mybir.AluOpType.mult,
                        mybir.AluOpType.add,
                    )
                    mt4 = m
                xy = xys[g]
                mtk = mt4[:, L * j : L * j + L]
                xcol = xy[:, 2 * j : 2 * j + 1]
                y0 = xy[0:1, 2 * j + 1 : 2 * j + 2]

                # bf16 weights: msc[s, t] = x_s * M[t, s]
                msc = mscpool.tile([L, L], bf16, tag="msc")
                nc.scalar.activation(msc[:], mtk, Copy, scale=xcol)

                if k > 0:
                    # p_t = prod_{r=block_start..t} y_r = y_0 * mt[0, t]
                    prow = rowpool.tile([1, L], f32, tag="prow")
                    nc.vector.tensor_scalar_mul(prow[:], mtk[0:1, :], y0)
                    # sel[s, t] = I[s==127] * p_t
                    pb = pbpool.tile([L, L], f32, tag="pb")
                    nc.gpsimd.partition_broadcast(pb[:], prow[0:1, :])
                    sel = selpool.tile([L, L], bf16, tag="sel")
                    nc.scalar.activation(sel[:], pb[:], Copy, scale=e127c[:])
                    sels[k] = sel

                po = ps_out_pool.tile([L, U], f32, tag="po")
                zt = zts[g]
                for jj in (0, 512):
                    nc.tensor.matmul(
                        po[:, jj : jj + 512],
                        msc[:],
                        zt[:, j * U + jj : j * U + jj + 512],
                        start=True,
                        stop=(k == 0),
                    )
                pos[k] = po

            def back(k: int):
                # carry accumulation + drain + output for block k
                po = pos.pop(k)
                if k > 0:
                    # po[t, :] += p_t * prev[127, :]
                    sel = sels.pop(k)
                    pt, pc = otbs[k - 1]
                    for jj in (0, 512):
                        nc.tensor.matmul(
                            po[:, jj : jj + 512],
                            sel[:],
                            pt[:, pc + jj : pc + jj + 512],
                            start=False,
                            stop=True,
                        )
                    otbs.pop(k - 1, None)
                # single bf16 drain, split DVE/Act; OB blocks share one
                # otb tile -> one batched output DMA
                h = k % OB
                if h == 0:
                    otb = otbpool.tile([L, OB * U], bf16, tag="otb")
                    otbs["cur"] = otb
                otb = otbs["cur"]
                c0 = h * U
                nc.vector.tensor_copy(otb[:, c0 : c0 + 512], po[:, 0:512])
                nc.scalar.copy(otb[:, c0 + 512 : c0 + U], po[:, 512:U])
                otbs[k] = (otb, c0)
                if h == OB - 1:
                    r0b = (k - OB + 1) * L
                    nc.sync.dma_start(
                        out[r0b : r0b + OB * L, :].rearrange(
                            "(a b) c -> b a c", a=OB
                        ),
                        otb[:],
                    )

            # software pipeline: PE runs block k's mains while block k-1's
            # carry waits on the drain, so the tensor engine never idles
            front(0)
            for k in range(1, nb):
                front(k)
                back(k - 1)
            back(nb - 1)
    nc.finalize()
    return nc


_NC = None


def _get_nc() -> bass.Bass:
    global _NC
    if _NC is None:
        _NC = build_nc()
    return _NC


def kernel(**inputs: np.ndarray) -> np.ndarray:
    x = np.ascontiguousarray(inputs["inputs"], dtype=np.float32)
    assert x.shape == (B, T, F), x.shape
    nc = _get_nc()
    in_maps = [{"inp": x[c]} for c in range(B)]
    res = run_bass_kernel_spmd(nc, in_maps, core_ids=list(range(B)))
    return np.stack(
        [np.asarray(res.results[c]["out"]).astype(np.float32) for c in range(B)],
        axis=0,
    )
